# revision 21
# baseline (speedup 1.0000x reference)
"""Multi-head attention (B=4, N=2048, D=1024, H=16) on 8 TRN2 NeuronCores.

Sharding: 8 cores = batch(4) x sequence-half(2). Each core computes the full
attention output for its 1024-token slice of one batch (all 16 heads); the
final unshard is a pure gather. Cross-core traffic: AllGather of K^T and V
between the two cores of each batch pair.

v2 vs baseline:
  - Host pre-transposes + pre-casts x/w_qkv/w_proj to bf16 tiles, so the
    on-device prologue is plain DMA loads (no fp32 load/cast/DRAM-roundtrip/
    DMA-transpose chain).
  - Denominator matmuls eliminated: V tiles carry ones columns per head
    pair ([V_even|1|1|V_odd], 130 cols). The O matmuls produce the softmax
    denominator as an extra PSUM partition row (even head: 65-wide lhsT,
    denom at partition 64; odd head: 128-wide lhsT window offset +2, denom
    at partition 63, O at 64..127).
  - Normalization: DVE reciprocal of the denom row (same partition),
    gpsimd partition_broadcast to the head's 64 partitions (no PE/PSUM
    cost), DVE multiply into attout pair tiles.
  - Scalar engine runs ONLY exp (plus early weight loads); collectives and
    gathered loads sit on gpsimd; gathered-K loads on sync.
"""

import sys

for _p in ("/opt/trn_rl_repo",):
    if _p not in sys.path:
        sys.path.insert(0, _p)

import numpy as np
import ml_dtypes

import concourse.bass as bass
import concourse.mybir as mybir
import concourse.tile as tile
from concourse import bacc
from concourse.bass_utils import run_bass_kernel_spmd

B, N, D, H, HD = 4, 2048, 1024, 16, 64
SCALE = HD ** -0.5
NL = N // 2  # tokens per core
NCORES = 8
RG = [[0, 1], [2, 3], [4, 5], [6, 7]]
F32 = mybir.dt.float32
BF16 = mybir.dt.bfloat16
EXP = mybir.ActivationFunctionType.Exp
BF = ml_dtypes.bfloat16


def _emit(tc, aps):
    nc = tc.nc
    xT_in, wqkvT_in, wpT_in, bias, out = (
        aps["xT_in"], aps["wqkvT_in"], aps["wpT_in"], aps["b_proj"], aps["out"])
    cc_kA, cc_kB, cc_vA, cc_vB = (aps["cc_kA"], aps["cc_kB"],
                                  aps["cc_vA"], aps["cc_vB"])
    k_gA, k_gB, v_gA, v_gB = aps["k_gA"], aps["k_gB"], aps["v_gA"], aps["v_gB"]
    rc_d = aps["rc_d"]

    persist = tc.alloc_tile_pool(name="persist", bufs=1)
    qkvp = tc.alloc_tile_pool(name="qkvp", bufs=1)
    qkvkv = tc.alloc_tile_pool(name="qkvkv", bufs=1)

    bias_sb = persist.tile([128, D], F32, tag="bias")
    bias_bcast = bass.AP(tensor=bias.tensor, offset=bias.offset,
                         ap=[[0, 128], *bias.ap])
    nc.sync.dma_start(out=bias_sb, in_=bias_bcast)

    # ---- input loads (all pre-transposed bf16 from host) ------------------
    xT = [qkvp.tile([128, NL], BF16, tag=f"xT{k}", name=f"xT{k}") for k in range(8)]
    # w_qkv^T split: K+V d_out cols (released after V proj) / Q cols (kept
    # through the attention loop for the interleaved Q projection)
    wTkv = [qkvkv.tile([128, 2 * D], BF16, tag=f"wTkv{k}", name=f"wTkv{k}")
            for k in range(8)]
    wTq = [qkvp.tile([128, D], BF16, tag=f"wTq{k}", name=f"wTq{k}")
           for k in range(8)]
    wpT = [persist.tile([128, D], BF16, tag=f"wpT{k}", name=f"wpT{k}")
           for k in range(8)]
    kT = [persist.tile([128, N], BF16, tag=f"kT{p}", name=f"kT{p}") for p in range(8)]
    qT = [persist.tile([128, NL], BF16, tag=f"qT{p}", name=f"qT{p}") for p in range(8)]
    # V pair-block layout per k-tile: 8 blocks of 160 cols:
    #   [V_even(64) | ones(2) | zeros(30) | V_odd(64)]
    # Even head lhsT = cols 0:65  -> O at psum parts 0..63, denom at 64.
    # Odd head lhsT = cols 32:160 -> junk at parts 0..31, denom at 32 (and
    # 33), zeros at 34..63, O at 64..127. Denoms land on 32-aligned
    # partitions as the engines require.
    vvA = [persist.tile([128, 8 * 160], BF16, tag=f"vvA{kt}", name=f"vvA{kt}")
           for kt in range(16)]

    # sync queue: all input loads in need-order (x + w_qkv K-rows first,
    # then V/Q rows, w_proj), later the gathered-K loads. The scalar DMA
    # ring stays EMPTY so the cc_v stores there issue without queuing
    # behind load transfers.
    for k in range(8):
        eng = nc.sync if k % 2 == 0 else nc.gpsimd
        eng.dma_start(out=xT[k], in_=xT_in[k])
    for k in range(8):
        eng = nc.sync if k % 2 == 1 else nc.gpsimd
        eng.dma_start(out=wTkv[k][:, 0:D], in_=wqkvT_in[k, :, D:2 * D])
    for k in range(8):
        nc.sync.dma_start(out=wTkv[k][:, D:2 * D], in_=wqkvT_in[k, :, 2 * D:3 * D])
    for k in range(8):
        nc.sync.dma_start(out=wTq[k], in_=wqkvT_in[k, :, 0:D])
    for k in range(8):
        nc.sync.dma_start(out=wpT[k], in_=wpT_in[k])

    # ones + zero columns of the V pair blocks (cols 64:66 / 66:96)
    for kt in range(16):
        blk = vvA[kt].rearrange("p (j c) -> p j c", j=8)
        nc.vector.memset(blk[:, :, 64:66], 1.0)
        nc.vector.memset(blk[:, :, 66:96], 0.0)

    with tc.tile_pool(name="qkvsb", bufs=2) as qkvsb, \
         tc.tile_pool(name="qkv_ps", bufs=2, space="PSUM") as qkvps:

        def proj_k(m, dst_sb):
            ps = qkvps.tile([128, 2, 512], F32, tag="qkv_ps")
            for k in range(8):
                for qc in range(2):
                    nc.tensor.matmul(
                        out=ps[:, qc, :],
                        lhsT=wTkv[k][:, (m - 8) * 128:(m - 7) * 128],
                        rhs=xT[k][:, qc * 512:(qc + 1) * 512],
                        start=(k == 0), stop=(k == 7))
            for qc in range(2):
                nc.vector.tensor_copy(dst_sb[:, qc * 512:(qc + 1) * 512], ps[:, qc, :])

        # K projection first so the K AllGathers launch as early as possible.
        # Split into two half-gathers so early head-pairs' K arrives sooner.
        for m in range(8, 16):
            ksb = qkvsb.tile([128, NL], BF16, tag="k_loc")
            proj_k(m, ksb)
            cc = cc_kA if m < 12 else cc_kB
            nc.gpsimd.dma_start(out=cc[(m % 4) * 128:(m % 4 + 1) * 128, :], in_=ksb)
            if m == 11:
                nc.gpsimd.collective_compute(
                    "AllGather", mybir.AluOpType.bypass, replica_groups=RG,
                    ins=[cc_kA], outs=[k_gA])
        nc.gpsimd.collective_compute(
            "AllGather", mybir.AluOpType.bypass, replica_groups=RG,
            ins=[cc_kB], outs=[k_gB])
        # gathered K loads on sync (block only on the K collectives)
        for p in range(8):
            g = k_gA if p < 4 else k_gB
            r = p % 4
            nc.sync.dma_start(out=kT[p][:, 0:NL], in_=g[0, r * 128:(r + 1) * 128, :])
            nc.sync.dma_start(out=kT[p][:, NL:N], in_=g[1, r * 128:(r + 1) * 128, :])

        # V projection (natural [token, d] orientation)
        for t in range(8):
            vsb = qkvsb.tile([128, D], BF16, tag="v_loc")
            ps = qkvps.tile([128, 2, 512], F32, tag="qkv_ps")
            for k in range(8):
                for vc in range(2):
                    nc.tensor.matmul(
                        out=ps[:, vc, :],
                        lhsT=xT[k][:, t * 128:(t + 1) * 128],
                        rhs=wTkv[k][:, D + vc * 512:D + (vc + 1) * 512],
                        start=(k == 0), stop=(k == 7))
            for vc in range(2):
                nc.vector.tensor_copy(vsb[:, vc * 512:(vc + 1) * 512], ps[:, vc, :])
            ccv = cc_vA if t < 4 else cc_vB
            nc.scalar.dma_start(out=ccv[(t % 4) * 128:(t % 4 + 1) * 128, :], in_=vsb)
            if t == 3:
                nc.gpsimd.collective_compute(
                    "AllGather", mybir.AluOpType.bypass, replica_groups=RG,
                    ins=[cc_vA], outs=[v_gA])

        # gathered V loads into pair-block layout (even cols 0:64 of each
        # 160-block, odd cols 96:160)
        def vva_load(kt):
            g = v_gA if (kt % 8) < 4 else v_gB
            src = g[kt // 8, (kt % 4) * 128:(kt % 4 + 1) * 128, :]
            src3 = src.rearrange("t (j two c) -> t j two c", j=8, two=2)
            dst3 = vvA[kt].rearrange("p (j c) -> p j c", j=8)
            nc.gpsimd.dma_start(out=dst3[:, :, 0:64], in_=src3[:, :, 0, :])
            nc.gpsimd.dma_start(out=dst3[:, :, 96:160], in_=src3[:, :, 1, :])

        for kt in (0, 1, 2, 3, 8, 9, 10, 11):
            vva_load(kt)
        nc.gpsimd.collective_compute(
            "AllGather", mybir.AluOpType.bypass, replica_groups=RG,
            ins=[cc_vB], outs=[v_gB])
        for kt in (4, 5, 6, 7, 12, 13, 14, 15):
            vva_load(kt)

    qkvkv.release()

    # ---- attention --------------------------------------------------------
    attout = [persist.tile([128, NL], BF16, tag=f"ao{p}", name=f"ao{p}")
              for p in range(8)]

    with tc.tile_pool(name="att_s", bufs=2, space="PSUM") as spool, \
         tc.tile_pool(name="att_o", bufs=2, space="PSUM") as opool, \
         tc.tile_pool(name="pT", bufs=4) as ppool, \
         tc.tile_pool(name="rc", bufs=2) as rcpool, \
         tc.tile_pool(name="rcb", bufs=2) as rcbpool:
        for p in range(8):
            # Q projection m-tile p (PSUM borrowed from the s pool), so exp
            # work starts ~80us earlier than a separate Q-proj phase
            qps = spool.tile([128, 2, 512], F32, tag="s_ps")
            for k in range(8):
                for qc in range(2):
                    nc.tensor.matmul(
                        out=qps[:, qc, :],
                        lhsT=wTq[k][:, p * 128:(p + 1) * 128],
                        rhs=xT[k][:, qc * 512:(qc + 1) * 512],
                        start=(k == 0), stop=(k == 7))
            for qc in range(2):
                nc.vector.tensor_copy(qT[p][:, qc * 512:(qc + 1) * 512],
                                      qps[:, qc, :])
            for qc in range(2):
                o = opool.tile([128, 2, 512], F32, tag="o_ps")
                for kt in range(16):
                    s = spool.tile([128, 2, 512], F32, tag="s_ps")
                    for h in range(2):
                        nc.tensor.matmul(
                            out=s[:, h, :],
                            lhsT=kT[p][h * 64:(h + 1) * 64, kt * 128:(kt + 1) * 128],
                            rhs=qT[p][h * 64:(h + 1) * 64, qc * 512:(qc + 1) * 512],
                            start=True, stop=True,
                            tile_position=(h * 64, 0))
                    pt = ppool.tile([128, 2, 512], BF16, tag="pT")
                    nc.scalar.activation(pt, s, EXP, scale=SCALE)
                    # even head: 128-col window -> O at parts 0..63, den at
                    # 64 (65: den copy; 66..95 zeros; 96..127 junk, unread).
                    # Full-width lhsT avoids a slow 65-partition matmul mode.
                    nc.tensor.matmul(
                        out=o[0:128, 0, :],
                        lhsT=vvA[kt][:, p * 160:p * 160 + 128],
                        rhs=pt[:, 0, :],
                        start=(kt == 0), stop=(kt == 15))
                    # odd head: 128-col window offset +32 -> junk 0..31,
                    # den at 32, zeros 34..63, O at 64..127
                    nc.tensor.matmul(
                        out=o[0:128, 1, :],
                        lhsT=vvA[kt][:, p * 160 + 32:(p + 1) * 160],
                        rhs=pt[:, 1, :],
                        start=(kt == 0), stop=(kt == 15))
                rc = rcpool.tile([128, 2, 512], F32, tag="rc")
                rcb = rcbpool.tile([128, 2, 512], F32, tag="rcb")
                nc.vector.reciprocal(rc[64:65, 0, :], o[64:65, 0, :])
                nc.vector.reciprocal(rc[32:33, 1, :], o[32:33, 1, :])
                # broadcast the reciprocal rows across the head's 64
                # partitions via a DRAM round-trip (partition-stride-0 load)
                for h, prow in ((0, 64), (1, 32)):
                    sl = rc_d[p, qc, h]
                    nc.gpsimd.dma_start(out=sl, in_=rc[prow:prow + 1, h, :])
                    bsrc = bass.AP(tensor=sl.tensor, offset=sl.offset,
                                   ap=[[0, 64], *sl.ap])
                    nc.gpsimd.dma_start(
                        out=rcb[h * 64:(h + 1) * 64, h, :], in_=bsrc)
                nc.vector.tensor_mul(
                    attout[p][0:64, qc * 512:(qc + 1) * 512],
                    o[0:64, 0, :], rcb[0:64, 0, :])
                nc.vector.tensor_mul(
                    attout[p][64:128, qc * 512:(qc + 1) * 512],
                    o[64:128, 1, :], rcb[64:128, 1, :])

    qkvp.release()

    # ---- output projection + bias -----------------------------------------
    with tc.tile_pool(name="proj_ps", bufs=2, space="PSUM") as projps, \
         tc.tile_pool(name="y_sb", bufs=3) as ypool:
        for tt in range(8):
            ps = projps.tile([128, 2, 512], F32, tag="proj_ps")
            for p in range(8):
                for ec in range(2):
                    nc.tensor.matmul(
                        out=ps[:, ec, :],
                        lhsT=attout[p][:, tt * 128:(tt + 1) * 128],
                        rhs=wpT[p][:, ec * 512:(ec + 1) * 512],
                        start=(p == 0), stop=(p == 7))
            yt = ypool.tile([128, D], F32, tag="y_sb")
            for ec in range(2):
                nc.vector.tensor_add(yt[:, ec * 512:(ec + 1) * 512], ps[:, ec, :],
                                     bias_sb[:, ec * 512:(ec + 1) * 512])
            nc.sync.dma_start(out=out[tt * 128:(tt + 1) * 128, :], in_=yt)
    persist.release()


def _build():
    nc = bacc.Bacc("TRN2", target_bir_lowering=False, debug=False,
                   num_devices=NCORES)
    aps = {
        "xT_in": nc.dram_tensor("xT_in", [8, 128, NL], BF16,
                                kind="ExternalInput").ap(),
        "wqkvT_in": nc.dram_tensor("wqkvT_in", [8, 128, 3 * D], BF16,
                                   kind="ExternalInput").ap(),
        "wpT_in": nc.dram_tensor("wpT_in", [8, 128, D], BF16,
                                 kind="ExternalInput").ap(),
        "b_proj": nc.dram_tensor("b_proj", [D], F32, kind="ExternalInput").ap(),
        "out": nc.dram_tensor("out", [NL, D], F32, kind="ExternalOutput").ap(),
        "cc_kA": nc.dram_tensor("cc_kA", [512, NL], BF16).ap(),
        "cc_kB": nc.dram_tensor("cc_kB", [512, NL], BF16).ap(),
        "cc_vA": nc.dram_tensor("cc_vA", [512, D], BF16).ap(),
        "cc_vB": nc.dram_tensor("cc_vB", [512, D], BF16).ap(),
        "k_gA": nc.dram_tensor("k_gA", [2, 512, NL], BF16).ap(),
        "k_gB": nc.dram_tensor("k_gB", [2, 512, NL], BF16).ap(),
        "v_gA": nc.dram_tensor("v_gA", [2, 512, D], BF16).ap(),
        "v_gB": nc.dram_tensor("v_gB", [2, 512, D], BF16).ap(),
        "rc_d": nc.dram_tensor("rc_d", [8, 2, 2, 512], F32).ap(),
    }
    with tile.TileContext(nc) as tc:
        _emit(tc, aps)
    nc.compile()
    return nc


_NC = None


def _get_nc():
    global _NC
    if _NC is None:
        _NC = _build()
    return _NC


def run(x, w_qkv, w_proj, b_proj, **spmd_kwargs):
    nc = _get_nc()
    x = np.asarray(x, dtype=np.float32)
    w_qkv = np.asarray(w_qkv, dtype=np.float32)
    w_proj = np.asarray(w_proj, dtype=np.float32)
    b_proj = np.ascontiguousarray(np.asarray(b_proj, dtype=np.float32))
    wqkvT = np.ascontiguousarray(
        w_qkv.T.reshape(8, 128, 3 * D).astype(BF))
    wpT = np.ascontiguousarray(
        w_proj.T.reshape(8, 128, D).astype(BF))
    in_maps = []
    for c in range(NCORES):
        b, half = divmod(c, 2)
        xT = np.ascontiguousarray(
            x[b, half * NL:(half + 1) * NL, :].T.reshape(8, 128, NL).astype(BF))
        in_maps.append({
            "xT_in": xT,
            "wqkvT_in": wqkvT,
            "wpT_in": wpT,
            "b_proj": b_proj,
        })
    res = run_bass_kernel_spmd(nc, in_maps, list(range(NCORES)), **spmd_kwargs)
    y = np.empty((B, N, D), dtype=np.float32)
    for c in range(NCORES):
        b, half = divmod(c, 2)
        y[b, half * NL:(half + 1) * NL, :] = res.results[c]["out"]
    return y, res


def kernel(x, w_qkv, w_proj, b_proj):
    y, _ = run(x, w_qkv, w_proj, b_proj)
    return y


# revision 23
# speedup vs baseline: 1.1251x; 1.1251x over previous
"""Multi-head attention (B=4, N=2048, D=1024, H=16) on 8 TRN2 NeuronCores.

Sharding: 8 cores = batch(4) x sequence-half(2). Each core computes the full
attention output for its 1024-token slice of one batch (all 16 heads); the
final unshard is a pure gather. Cross-core traffic: AllGather of K^T and V
between the two cores of each batch pair.

v2 vs baseline:
  - Host pre-transposes + pre-casts x/w_qkv/w_proj to bf16 tiles, so the
    on-device prologue is plain DMA loads (no fp32 load/cast/DRAM-roundtrip/
    DMA-transpose chain).
  - Denominator matmuls eliminated: V tiles carry ones columns per head
    pair ([V_even|1|1|V_odd], 130 cols). The O matmuls produce the softmax
    denominator as an extra PSUM partition row (even head: 65-wide lhsT,
    denom at partition 64; odd head: 128-wide lhsT window offset +2, denom
    at partition 63, O at 64..127).
  - Normalization: DVE reciprocal of the denom row (same partition),
    gpsimd partition_broadcast to the head's 64 partitions (no PE/PSUM
    cost), DVE multiply into attout pair tiles.
  - Scalar engine runs ONLY exp (plus early weight loads); collectives and
    gathered loads sit on gpsimd; gathered-K loads on sync.
"""

import sys

for _p in ("/opt/trn_rl_repo",):
    if _p not in sys.path:
        sys.path.insert(0, _p)

import numpy as np
import ml_dtypes

import concourse.bass as bass
import concourse.mybir as mybir
import concourse.tile as tile
from concourse import bacc
from concourse.bass_utils import run_bass_kernel_spmd

B, N, D, H, HD = 4, 2048, 1024, 16, 64
SCALE = HD ** -0.5
NL = N // 2  # tokens per core
NCORES = 8
RG = [[0, 1], [2, 3], [4, 5], [6, 7]]
F32 = mybir.dt.float32
BF16 = mybir.dt.bfloat16
EXP = mybir.ActivationFunctionType.Exp
BF = ml_dtypes.bfloat16


def _emit(tc, aps):
    nc = tc.nc
    xT_in, wqkvT_in, wpT_in, bias, out = (
        aps["xT_in"], aps["wqkvT_in"], aps["wpT_in"], aps["b_proj"], aps["out"])
    cc_kA, cc_kB, cc_vA, cc_vB = (aps["cc_kA"], aps["cc_kB"],
                                  aps["cc_vA"], aps["cc_vB"])
    k_gA, k_gB, v_gA, v_gB = aps["k_gA"], aps["k_gB"], aps["v_gA"], aps["v_gB"]
    rc_d = aps["rc_d"]

    persist = tc.alloc_tile_pool(name="persist", bufs=1)
    qkvp = tc.alloc_tile_pool(name="qkvp", bufs=1)
    qkvkv = tc.alloc_tile_pool(name="qkvkv", bufs=1)

    bias_sb = persist.tile([128, D], F32, tag="bias")
    bias_bcast = bass.AP(tensor=bias.tensor, offset=bias.offset,
                         ap=[[0, 128], *bias.ap])
    nc.sync.dma_start(out=bias_sb, in_=bias_bcast)

    # ---- input loads (all pre-transposed bf16 from host) ------------------
    xT = [qkvp.tile([128, NL], BF16, tag=f"xT{k}", name=f"xT{k}") for k in range(8)]
    # w_qkv^T split: K+V d_out cols (released after V proj) / Q cols (kept
    # through the attention loop for the interleaved Q projection)
    wTkv = [qkvkv.tile([128, 2 * D], BF16, tag=f"wTkv{k}", name=f"wTkv{k}")
            for k in range(8)]
    wTq = [qkvp.tile([128, D], BF16, tag=f"wTq{k}", name=f"wTq{k}")
           for k in range(8)]
    wpT = [persist.tile([128, D], BF16, tag=f"wpT{k}", name=f"wpT{k}")
           for k in range(8)]
    kT = [persist.tile([128, N], BF16, tag=f"kT{p}", name=f"kT{p}") for p in range(8)]
    qT = [persist.tile([128, NL], BF16, tag=f"qT{p}", name=f"qT{p}") for p in range(8)]
    # V pair-block layout per k-tile: 8 blocks of 160 cols:
    #   [V_even(64) | ones(2) | zeros(30) | V_odd(64)]
    # Even head lhsT = cols 0:65  -> O at psum parts 0..63, denom at 64.
    # Odd head lhsT = cols 32:160 -> junk at parts 0..31, denom at 32 (and
    # 33), zeros at 34..63, O at 64..127. Denoms land on 32-aligned
    # partitions as the engines require.
    vvA = [persist.tile([128, 8 * 160], BF16, tag=f"vvA{kt}", name=f"vvA{kt}")
           for kt in range(16)]

    # sync queue: all input loads in need-order (x + w_qkv K-rows first,
    # then V/Q rows, w_proj), later the gathered-K loads. The scalar DMA
    # ring stays EMPTY so the cc_v stores there issue without queuing
    # behind load transfers.
    for k in range(8):
        eng = nc.sync if k % 2 == 0 else nc.gpsimd
        eng.dma_start(out=xT[k], in_=xT_in[k])
    for k in range(8):
        eng = nc.sync if k % 2 == 1 else nc.gpsimd
        eng.dma_start(out=wTkv[k][:, 0:D], in_=wqkvT_in[k, :, D:2 * D])
    for k in range(8):
        nc.sync.dma_start(out=wTkv[k][:, D:2 * D], in_=wqkvT_in[k, :, 2 * D:3 * D])
    for k in range(8):
        nc.sync.dma_start(out=wTq[k], in_=wqkvT_in[k, :, 0:D])
    for k in range(8):
        nc.sync.dma_start(out=wpT[k], in_=wpT_in[k])

    # ones + zero columns of the V pair blocks (cols 64:66 / 66:96)
    for kt in range(16):
        blk = vvA[kt].rearrange("p (j c) -> p j c", j=8)
        nc.vector.memset(blk[:, :, 64:66], 1.0)
        nc.vector.memset(blk[:, :, 66:96], 0.0)

    with tc.tile_pool(name="qkvsb", bufs=2) as qkvsb, \
         tc.tile_pool(name="qkv_ps", bufs=2, space="PSUM") as qkvps:

        def proj_k(m, dst_sb):
            ps = qkvps.tile([128, 2, 512], F32, tag="qkv_ps")
            for k in range(8):
                for qc in range(2):
                    nc.tensor.matmul(
                        out=ps[:, qc, :],
                        lhsT=wTkv[k][:, (m - 8) * 128:(m - 7) * 128],
                        rhs=xT[k][:, qc * 512:(qc + 1) * 512],
                        start=(k == 0), stop=(k == 7))
            for qc in range(2):
                nc.vector.tensor_copy(dst_sb[:, qc * 512:(qc + 1) * 512], ps[:, qc, :])

        # K projection first so the K AllGathers launch as early as possible.
        # Split into two half-gathers so early head-pairs' K arrives sooner.
        for m in range(8, 16):
            ksb = qkvsb.tile([128, NL], BF16, tag="k_loc")
            proj_k(m, ksb)
            cc = cc_kA if m < 12 else cc_kB
            nc.gpsimd.dma_start(out=cc[(m % 4) * 128:(m % 4 + 1) * 128, :], in_=ksb)
            if m == 11:
                nc.gpsimd.collective_compute(
                    "AllGather", mybir.AluOpType.bypass, replica_groups=RG,
                    ins=[cc_kA], outs=[k_gA])
        nc.gpsimd.collective_compute(
            "AllGather", mybir.AluOpType.bypass, replica_groups=RG,
            ins=[cc_kB], outs=[k_gB])
        # gathered K loads on sync (block only on the K collectives)
        for p in range(8):
            g = k_gA if p < 4 else k_gB
            r = p % 4
            nc.sync.dma_start(out=kT[p][:, 0:NL], in_=g[0, r * 128:(r + 1) * 128, :])
            nc.sync.dma_start(out=kT[p][:, NL:N], in_=g[1, r * 128:(r + 1) * 128, :])

        # V projection (natural [token, d] orientation)
        for t in range(8):
            vsb = qkvsb.tile([128, D], BF16, tag="v_loc")
            ps = qkvps.tile([128, 2, 512], F32, tag="qkv_ps")
            for k in range(8):
                for vc in range(2):
                    nc.tensor.matmul(
                        out=ps[:, vc, :],
                        lhsT=xT[k][:, t * 128:(t + 1) * 128],
                        rhs=wTkv[k][:, D + vc * 512:D + (vc + 1) * 512],
                        start=(k == 0), stop=(k == 7))
            for vc in range(2):
                nc.vector.tensor_copy(vsb[:, vc * 512:(vc + 1) * 512], ps[:, vc, :])
            ccv = cc_vA if t < 4 else cc_vB
            nc.scalar.dma_start(out=ccv[(t % 4) * 128:(t % 4 + 1) * 128, :], in_=vsb)
            if t == 3:
                nc.gpsimd.collective_compute(
                    "AllGather", mybir.AluOpType.bypass, replica_groups=RG,
                    ins=[cc_vA], outs=[v_gA])

        # gathered V loads into pair-block layout (even cols 0:64 of each
        # 160-block, odd cols 96:160)
        def vva_load(kt):
            g = v_gA if (kt % 8) < 4 else v_gB
            src = g[kt // 8, (kt % 4) * 128:(kt % 4 + 1) * 128, :]
            src3 = src.rearrange("t (j two c) -> t j two c", j=8, two=2)
            dst3 = vvA[kt].rearrange("p (j c) -> p j c", j=8)
            nc.gpsimd.dma_start(out=dst3[:, :, 0:64], in_=src3[:, :, 0, :])
            nc.gpsimd.dma_start(out=dst3[:, :, 96:160], in_=src3[:, :, 1, :])

        for kt in (0, 1, 2, 3, 8, 9, 10, 11):
            vva_load(kt)
        nc.gpsimd.collective_compute(
            "AllGather", mybir.AluOpType.bypass, replica_groups=RG,
            ins=[cc_vB], outs=[v_gB])
        for kt in (4, 5, 6, 7, 12, 13, 14, 15):
            vva_load(kt)

        # Q projection (overlaps the V gathers; attention starts right after)
        for m in range(8):
            ps = qkvps.tile([128, 2, 512], F32, tag="qkv_ps")
            for k in range(8):
                for qc in range(2):
                    nc.tensor.matmul(
                        out=ps[:, qc, :],
                        lhsT=wTq[k][:, m * 128:(m + 1) * 128],
                        rhs=xT[k][:, qc * 512:(qc + 1) * 512],
                        start=(k == 0), stop=(k == 7))
            for qc in range(2):
                nc.vector.tensor_copy(qT[m][:, qc * 512:(qc + 1) * 512],
                                      ps[:, qc, :])

    qkvkv.release()

    # ---- attention --------------------------------------------------------
    attout = [persist.tile([128, NL], BF16, tag=f"ao{p}", name=f"ao{p}")
              for p in range(8)]

    with tc.tile_pool(name="att_s", bufs=2, space="PSUM") as spool, \
         tc.tile_pool(name="att_o", bufs=2, space="PSUM") as opool, \
         tc.tile_pool(name="pT", bufs=4) as ppool, \
         tc.tile_pool(name="rc", bufs=2) as rcpool:
        for p in range(8):
            for qc in range(2):
                o = opool.tile([128, 2, 512], F32, tag="o_ps")
                for kt in range(16):
                    s = spool.tile([128, 2, 512], F32, tag="s_ps")
                    for h in range(2):
                        nc.tensor.matmul(
                            out=s[:, h, :],
                            lhsT=kT[p][h * 64:(h + 1) * 64, kt * 128:(kt + 1) * 128],
                            rhs=qT[p][h * 64:(h + 1) * 64, qc * 512:(qc + 1) * 512],
                            start=True, stop=True,
                            tile_position=(h * 64, 0))
                    pt = ppool.tile([128, 2, 512], BF16, tag="pT")
                    nc.scalar.activation(pt, s, EXP, scale=SCALE)
                    # even head: 128-col window -> O at parts 0..63, den at
                    # 64 (65: den copy; 66..95 zeros; 96..127 junk, unread).
                    # Full-width lhsT avoids a slow 65-partition matmul mode.
                    nc.tensor.matmul(
                        out=o[0:128, 0, :],
                        lhsT=vvA[kt][:, p * 160:p * 160 + 128],
                        rhs=pt[:, 0, :],
                        start=(kt == 0), stop=(kt == 15))
                    # odd head: 128-col window offset +32 -> junk 0..31,
                    # den at 32, zeros 34..63, O at 64..127
                    nc.tensor.matmul(
                        out=o[0:128, 1, :],
                        lhsT=vvA[kt][:, p * 160 + 32:(p + 1) * 160],
                        rhs=pt[:, 1, :],
                        start=(kt == 0), stop=(kt == 15))
                # rc rows 64 (h0) / 32 (h1) hold the reciprocals; the
                # broadcasts land at parts 0:64 (h0) / 64:128 (h1) of the
                # same tile -- disjoint regions, one tile does both jobs
                rc = rcpool.tile([128, 2, 512], F32, tag="rc")
                rcb = rc
                nc.vector.reciprocal(rc[64:65, 0, :], o[64:65, 0, :])
                nc.vector.reciprocal(rc[32:33, 1, :], o[32:33, 1, :])
                # broadcast the reciprocal rows across the head's 64
                # partitions via a DRAM round-trip (partition-stride-0 load)
                for h, prow in ((0, 64), (1, 32)):
                    sl = rc_d[p, qc, h]
                    nc.gpsimd.dma_start(out=sl, in_=rc[prow:prow + 1, h, :])
                    bsrc = bass.AP(tensor=sl.tensor, offset=sl.offset,
                                   ap=[[0, 64], *sl.ap])
                    nc.gpsimd.dma_start(
                        out=rcb[h * 64:(h + 1) * 64, h, :], in_=bsrc)
                nc.vector.tensor_mul(
                    attout[p][0:64, qc * 512:(qc + 1) * 512],
                    o[0:64, 0, :], rcb[0:64, 0, :])
                nc.vector.tensor_mul(
                    attout[p][64:128, qc * 512:(qc + 1) * 512],
                    o[64:128, 1, :], rcb[64:128, 1, :])

    qkvp.release()

    # ---- output projection + bias -----------------------------------------
    with tc.tile_pool(name="proj_ps", bufs=2, space="PSUM") as projps, \
         tc.tile_pool(name="y_sb", bufs=3) as ypool:
        for tt in range(8):
            ps = projps.tile([128, 2, 512], F32, tag="proj_ps")
            for p in range(8):
                for ec in range(2):
                    nc.tensor.matmul(
                        out=ps[:, ec, :],
                        lhsT=attout[p][:, tt * 128:(tt + 1) * 128],
                        rhs=wpT[p][:, ec * 512:(ec + 1) * 512],
                        start=(p == 0), stop=(p == 7))
            yt = ypool.tile([128, D], BF16, tag="y_sb")
            for ec in range(2):
                nc.vector.tensor_add(yt[:, ec * 512:(ec + 1) * 512], ps[:, ec, :],
                                     bias_sb[:, ec * 512:(ec + 1) * 512])
            nc.sync.dma_start(out=out[tt * 128:(tt + 1) * 128, :], in_=yt)
    persist.release()


def _build():
    nc = bacc.Bacc("TRN2", target_bir_lowering=False, debug=False,
                   num_devices=NCORES)
    aps = {
        "xT_in": nc.dram_tensor("xT_in", [8, 128, NL], BF16,
                                kind="ExternalInput").ap(),
        "wqkvT_in": nc.dram_tensor("wqkvT_in", [8, 128, 3 * D], BF16,
                                   kind="ExternalInput").ap(),
        "wpT_in": nc.dram_tensor("wpT_in", [8, 128, D], BF16,
                                 kind="ExternalInput").ap(),
        "b_proj": nc.dram_tensor("b_proj", [D], F32, kind="ExternalInput").ap(),
        "out": nc.dram_tensor("out", [NL, D], BF16,
                              kind="ExternalOutput").ap(),
        "cc_kA": nc.dram_tensor("cc_kA", [512, NL], BF16).ap(),
        "cc_kB": nc.dram_tensor("cc_kB", [512, NL], BF16).ap(),
        "cc_vA": nc.dram_tensor("cc_vA", [512, D], BF16).ap(),
        "cc_vB": nc.dram_tensor("cc_vB", [512, D], BF16).ap(),
        "k_gA": nc.dram_tensor("k_gA", [2, 512, NL], BF16).ap(),
        "k_gB": nc.dram_tensor("k_gB", [2, 512, NL], BF16).ap(),
        "v_gA": nc.dram_tensor("v_gA", [2, 512, D], BF16).ap(),
        "v_gB": nc.dram_tensor("v_gB", [2, 512, D], BF16).ap(),
        "rc_d": nc.dram_tensor("rc_d", [8, 2, 2, 512], F32).ap(),
    }
    with tile.TileContext(nc) as tc:
        _emit(tc, aps)
    nc.compile()
    return nc


_NC = None


def _get_nc():
    global _NC
    if _NC is None:
        _NC = _build()
    return _NC


def run(x, w_qkv, w_proj, b_proj, **spmd_kwargs):
    nc = _get_nc()
    x = np.asarray(x, dtype=np.float32)
    w_qkv = np.asarray(w_qkv, dtype=np.float32)
    w_proj = np.asarray(w_proj, dtype=np.float32)
    b_proj = np.ascontiguousarray(np.asarray(b_proj, dtype=np.float32))
    wqkvT = np.ascontiguousarray(
        w_qkv.T.reshape(8, 128, 3 * D).astype(BF))
    wpT = np.ascontiguousarray(
        w_proj.T.reshape(8, 128, D).astype(BF))
    in_maps = []
    for c in range(NCORES):
        b, half = divmod(c, 2)
        xT = np.ascontiguousarray(
            x[b, half * NL:(half + 1) * NL, :].T.reshape(8, 128, NL).astype(BF))
        in_maps.append({
            "xT_in": xT,
            "wqkvT_in": wqkvT,
            "wpT_in": wpT,
            "b_proj": b_proj,
        })
    res = run_bass_kernel_spmd(nc, in_maps, list(range(NCORES)), **spmd_kwargs)
    y = np.empty((B, N, D), dtype=np.float32)
    for c in range(NCORES):
        b, half = divmod(c, 2)
        y[b, half * NL:(half + 1) * NL, :] = res.results[c]["out"].astype(
            np.float32)
    return y, res


def kernel(x, w_qkv, w_proj, b_proj):
    y, _ = run(x, w_qkv, w_proj, b_proj)
    return y


# revision 24
# speedup vs baseline: 1.1381x; 1.0116x over previous
"""Multi-head attention (B=4, N=2048, D=1024, H=16) on 8 TRN2 NeuronCores.

Sharding: 8 cores = batch(4) x sequence-half(2). Each core computes the full
attention output for its 1024-token slice of one batch (all 16 heads); the
final unshard is a pure gather. Cross-core traffic: AllGather of K^T and V
between the two cores of each batch pair.

v2 vs baseline:
  - Host pre-transposes + pre-casts x/w_qkv/w_proj to bf16 tiles, so the
    on-device prologue is plain DMA loads (no fp32 load/cast/DRAM-roundtrip/
    DMA-transpose chain).
  - Denominator matmuls eliminated: V tiles carry ones columns per head
    pair ([V_even|1|1|V_odd], 130 cols). The O matmuls produce the softmax
    denominator as an extra PSUM partition row (even head: 65-wide lhsT,
    denom at partition 64; odd head: 128-wide lhsT window offset +2, denom
    at partition 63, O at 64..127).
  - Normalization: DVE reciprocal of the denom row (same partition),
    gpsimd partition_broadcast to the head's 64 partitions (no PE/PSUM
    cost), DVE multiply into attout pair tiles.
  - Scalar engine runs ONLY exp (plus early weight loads); collectives and
    gathered loads sit on gpsimd; gathered-K loads on sync.
"""

import sys

for _p in ("/opt/trn_rl_repo",):
    if _p not in sys.path:
        sys.path.insert(0, _p)

import numpy as np
import ml_dtypes

import concourse.bass as bass
import concourse.mybir as mybir
import concourse.tile as tile
from concourse import bacc
from concourse.bass_utils import run_bass_kernel_spmd

B, N, D, H, HD = 4, 2048, 1024, 16, 64
SCALE = HD ** -0.5
NL = N // 2  # tokens per core
NCORES = 8
RG = [[0, 1], [2, 3], [4, 5], [6, 7]]
F32 = mybir.dt.float32
BF16 = mybir.dt.bfloat16
EXP = mybir.ActivationFunctionType.Exp
BF = ml_dtypes.bfloat16


def _emit(tc, aps):
    nc = tc.nc
    xT_in, wqkvT_in, wpT_in, bias, out = (
        aps["xT_in"], aps["wqkvT_in"], aps["wpT_in"], aps["b_proj"], aps["out"])
    cc_kA, cc_kB, cc_vA, cc_vB = (aps["cc_kA"], aps["cc_kB"],
                                  aps["cc_vA"], aps["cc_vB"])
    k_gA, k_gB, v_gA, v_gB = aps["k_gA"], aps["k_gB"], aps["v_gA"], aps["v_gB"]
    rc_d = aps["rc_d"]

    persist = tc.alloc_tile_pool(name="persist", bufs=1)
    qkvp = tc.alloc_tile_pool(name="qkvp", bufs=1)
    qkvkv = tc.alloc_tile_pool(name="qkvkv", bufs=1)

    bias_sb = persist.tile([128, D], F32, tag="bias")
    bias_bcast = bass.AP(tensor=bias.tensor, offset=bias.offset,
                         ap=[[0, 128], *bias.ap])
    nc.sync.dma_start(out=bias_sb, in_=bias_bcast)

    # ---- input loads (all pre-transposed bf16 from host) ------------------
    xT = [qkvp.tile([128, NL], BF16, tag=f"xT{k}", name=f"xT{k}") for k in range(8)]
    # w_qkv^T split: K+V d_out cols (released after V proj) / Q cols (kept
    # through the attention loop for the interleaved Q projection)
    wTkv = [qkvkv.tile([128, 2 * D], BF16, tag=f"wTkv{k}", name=f"wTkv{k}")
            for k in range(8)]
    wTq = [qkvp.tile([128, D], BF16, tag=f"wTq{k}", name=f"wTq{k}")
           for k in range(8)]
    wpT = [persist.tile([128, D], BF16, tag=f"wpT{k}", name=f"wpT{k}")
           for k in range(8)]
    kT = [persist.tile([128, N], BF16, tag=f"kT{p}", name=f"kT{p}") for p in range(8)]
    qT = [persist.tile([128, NL], BF16, tag=f"qT{p}", name=f"qT{p}") for p in range(8)]
    # V pair-block layout per k-tile: 8 blocks of 160 cols:
    #   [V_even(64) | ones(2) | zeros(30) | V_odd(64)]
    # Even head lhsT = cols 0:65  -> O at psum parts 0..63, denom at 64.
    # Odd head lhsT = cols 32:160 -> junk at parts 0..31, denom at 32 (and
    # 33), zeros at 34..63, O at 64..127. Denoms land on 32-aligned
    # partitions as the engines require.
    vvA = [persist.tile([128, 8 * 160], BF16, tag=f"vvA{kt}", name=f"vvA{kt}")
           for kt in range(16)]

    # sync queue: all input loads in need-order (x + w_qkv K-rows first,
    # then V/Q rows, w_proj), later the gathered-K loads. The scalar DMA
    # ring stays EMPTY so the cc_v stores there issue without queuing
    # behind load transfers.
    for k in range(8):
        eng = nc.sync if k % 2 == 0 else nc.gpsimd
        eng.dma_start(out=xT[k], in_=xT_in[k])
    for k in range(8):
        eng = nc.sync if k % 2 == 1 else nc.gpsimd
        eng.dma_start(out=wTkv[k][:, 0:D], in_=wqkvT_in[k, :, D:2 * D])
    for k in range(8):
        nc.sync.dma_start(out=wTkv[k][:, D:2 * D], in_=wqkvT_in[k, :, 2 * D:3 * D])
    for k in range(8):
        nc.sync.dma_start(out=wTq[k], in_=wqkvT_in[k, :, 0:D])
    for k in range(8):
        nc.sync.dma_start(out=wpT[k], in_=wpT_in[k])

    # warm up the activation table + its const operands at t~0 so the
    # hoisted ACT_TABLE_LOAD doesn't gate the scalar queue mid-kernel
    warm = persist.tile([128, 1], F32, tag="warm")
    nc.vector.memset(warm, 0.0)
    nc.scalar.activation(warm, warm, EXP, scale=SCALE)

    # ones + zero columns of the V pair blocks (cols 64:66 / 66:96)
    for kt in range(16):
        blk = vvA[kt].rearrange("p (j c) -> p j c", j=8)
        nc.vector.memset(blk[:, :, 64:66], 1.0)
        nc.vector.memset(blk[:, :, 66:96], 0.0)

    with tc.tile_pool(name="qkvsb", bufs=2) as qkvsb, \
         tc.tile_pool(name="qkv_ps", bufs=2, space="PSUM") as qkvps:

        def proj_k(m, dst_sb):
            ps = qkvps.tile([128, 2, 512], F32, tag="qkv_ps")
            for k in range(8):
                for qc in range(2):
                    nc.tensor.matmul(
                        out=ps[:, qc, :],
                        lhsT=wTkv[k][:, (m - 8) * 128:(m - 7) * 128],
                        rhs=xT[k][:, qc * 512:(qc + 1) * 512],
                        start=(k == 0), stop=(k == 7))
            for qc in range(2):
                nc.vector.tensor_copy(dst_sb[:, qc * 512:(qc + 1) * 512], ps[:, qc, :])

        # K projection first so the K AllGathers launch as early as possible.
        # Split into two half-gathers so early head-pairs' K arrives sooner.
        for m in range(8, 16):
            ksb = qkvsb.tile([128, NL], BF16, tag="k_loc")
            proj_k(m, ksb)
            cc = cc_kA if m < 12 else cc_kB
            nc.gpsimd.dma_start(out=cc[(m % 4) * 128:(m % 4 + 1) * 128, :], in_=ksb)
            if m == 11:
                nc.gpsimd.collective_compute(
                    "AllGather", mybir.AluOpType.bypass, replica_groups=RG,
                    ins=[cc_kA], outs=[k_gA])
        nc.gpsimd.collective_compute(
            "AllGather", mybir.AluOpType.bypass, replica_groups=RG,
            ins=[cc_kB], outs=[k_gB])

        # V projection (natural [token, d] orientation)
        for t in range(8):
            vsb = qkvsb.tile([128, D], BF16, tag="v_loc")
            ps = qkvps.tile([128, 2, 512], F32, tag="qkv_ps")
            for k in range(8):
                for vc in range(2):
                    nc.tensor.matmul(
                        out=ps[:, vc, :],
                        lhsT=xT[k][:, t * 128:(t + 1) * 128],
                        rhs=wTkv[k][:, D + vc * 512:D + (vc + 1) * 512],
                        start=(k == 0), stop=(k == 7))
            for vc in range(2):
                nc.vector.tensor_copy(vsb[:, vc * 512:(vc + 1) * 512], ps[:, vc, :])
            ccv = cc_vA if t < 4 else cc_vB
            nc.scalar.dma_start(out=ccv[(t % 4) * 128:(t % 4 + 1) * 128, :], in_=vsb)
            if t == 3:
                nc.gpsimd.collective_compute(
                    "AllGather", mybir.AluOpType.bypass, replica_groups=RG,
                    ins=[cc_vA], outs=[v_gA])

        # gathered V loads into pair-block layout (even cols 0:64 of each
        # 160-block, odd cols 96:160)
        def vva_load(kt):
            g = v_gA if (kt % 8) < 4 else v_gB
            src = g[kt // 8, (kt % 4) * 128:(kt % 4 + 1) * 128, :]
            src3 = src.rearrange("t (j two c) -> t j two c", j=8, two=2)
            dst3 = vvA[kt].rearrange("p (j c) -> p j c", j=8)
            nc.gpsimd.dma_start(out=dst3[:, :, 0:64], in_=src3[:, :, 0, :])
            nc.gpsimd.dma_start(out=dst3[:, :, 96:160], in_=src3[:, :, 1, :])

        for kt in (0, 1, 2, 3, 8, 9, 10, 11):
            vva_load(kt)
        nc.gpsimd.collective_compute(
            "AllGather", mybir.AluOpType.bypass, replica_groups=RG,
            ins=[cc_vB], outs=[v_gB])
        for kt in (4, 5, 6, 7, 12, 13, 14, 15):
            vva_load(kt)

        # gathered K loads on sync, emitted AFTER the V/ccV DMAs so no
        # V-phase DMA queues behind the K collectives
        for p in range(8):
            g = k_gA if p < 4 else k_gB
            r = p % 4
            nc.sync.dma_start(out=kT[p][:, 0:NL], in_=g[0, r * 128:(r + 1) * 128, :])
            nc.sync.dma_start(out=kT[p][:, NL:N], in_=g[1, r * 128:(r + 1) * 128, :])

        # Q projection (overlaps the V gathers; attention starts right after)
        for m in range(8):
            ps = qkvps.tile([128, 2, 512], F32, tag="qkv_ps")
            for k in range(8):
                for qc in range(2):
                    nc.tensor.matmul(
                        out=ps[:, qc, :],
                        lhsT=wTq[k][:, m * 128:(m + 1) * 128],
                        rhs=xT[k][:, qc * 512:(qc + 1) * 512],
                        start=(k == 0), stop=(k == 7))
            for qc in range(2):
                nc.vector.tensor_copy(qT[m][:, qc * 512:(qc + 1) * 512],
                                      ps[:, qc, :])

    qkvkv.release()

    # ---- attention --------------------------------------------------------
    attout = [persist.tile([128, NL], BF16, tag=f"ao{p}", name=f"ao{p}")
              for p in range(8)]

    with tc.tile_pool(name="att_s", bufs=2, space="PSUM") as spool, \
         tc.tile_pool(name="att_o", bufs=2, space="PSUM") as opool, \
         tc.tile_pool(name="pT", bufs=4) as ppool, \
         tc.tile_pool(name="rc", bufs=2) as rcpool:
        for p in range(8):
            for qc in range(2):
                o = opool.tile([128, 2, 512], F32, tag="o_ps")
                for kt in range(16):
                    s = spool.tile([128, 2, 512], F32, tag="s_ps")
                    for h in range(2):
                        nc.tensor.matmul(
                            out=s[:, h, :],
                            lhsT=kT[p][h * 64:(h + 1) * 64, kt * 128:(kt + 1) * 128],
                            rhs=qT[p][h * 64:(h + 1) * 64, qc * 512:(qc + 1) * 512],
                            start=True, stop=True,
                            tile_position=(h * 64, 0))
                    pt = ppool.tile([128, 2, 512], BF16, tag="pT")
                    nc.scalar.activation(pt, s, EXP, scale=SCALE)
                    # even head: 128-col window -> O at parts 0..63, den at
                    # 64 (65: den copy; 66..95 zeros; 96..127 junk, unread).
                    # Full-width lhsT avoids a slow 65-partition matmul mode.
                    nc.tensor.matmul(
                        out=o[0:128, 0, :],
                        lhsT=vvA[kt][:, p * 160:p * 160 + 128],
                        rhs=pt[:, 0, :],
                        start=(kt == 0), stop=(kt == 15))
                    # odd head: 128-col window offset +32 -> junk 0..31,
                    # den at 32, zeros 34..63, O at 64..127
                    nc.tensor.matmul(
                        out=o[0:128, 1, :],
                        lhsT=vvA[kt][:, p * 160 + 32:(p + 1) * 160],
                        rhs=pt[:, 1, :],
                        start=(kt == 0), stop=(kt == 15))
                # rc rows 64 (h0) / 32 (h1) hold the reciprocals; the
                # broadcasts land at parts 0:64 (h0) / 64:128 (h1) of the
                # same tile -- disjoint regions, one tile does both jobs
                rc = rcpool.tile([128, 2, 512], F32, tag="rc")
                rcb = rc
                nc.vector.reciprocal(rc[64:65, 0, :], o[64:65, 0, :])
                nc.vector.reciprocal(rc[32:33, 1, :], o[32:33, 1, :])
                # broadcast the reciprocal rows across the head's 64
                # partitions via a DRAM round-trip (partition-stride-0 load)
                for h, prow in ((0, 64), (1, 32)):
                    sl = rc_d[p, qc, h]
                    nc.gpsimd.dma_start(out=sl, in_=rc[prow:prow + 1, h, :])
                    bsrc = bass.AP(tensor=sl.tensor, offset=sl.offset,
                                   ap=[[0, 64], *sl.ap])
                    nc.gpsimd.dma_start(
                        out=rcb[h * 64:(h + 1) * 64, h, :], in_=bsrc)
                nc.vector.tensor_mul(
                    attout[p][0:64, qc * 512:(qc + 1) * 512],
                    o[0:64, 0, :], rcb[0:64, 0, :])
                nc.vector.tensor_mul(
                    attout[p][64:128, qc * 512:(qc + 1) * 512],
                    o[64:128, 1, :], rcb[64:128, 1, :])

    qkvp.release()

    # ---- output projection + bias -----------------------------------------
    with tc.tile_pool(name="proj_ps", bufs=2, space="PSUM") as projps, \
         tc.tile_pool(name="y_sb", bufs=3) as ypool:
        for tt in range(8):
            ps = projps.tile([128, 2, 512], F32, tag="proj_ps")
            for p in range(8):
                for ec in range(2):
                    nc.tensor.matmul(
                        out=ps[:, ec, :],
                        lhsT=attout[p][:, tt * 128:(tt + 1) * 128],
                        rhs=wpT[p][:, ec * 512:(ec + 1) * 512],
                        start=(p == 0), stop=(p == 7))
            yt = ypool.tile([128, D], BF16, tag="y_sb")
            for ec in range(2):
                nc.vector.tensor_add(yt[:, ec * 512:(ec + 1) * 512], ps[:, ec, :],
                                     bias_sb[:, ec * 512:(ec + 1) * 512])
            nc.sync.dma_start(out=out[tt * 128:(tt + 1) * 128, :], in_=yt)
    persist.release()


def _build():
    nc = bacc.Bacc("TRN2", target_bir_lowering=False, debug=False,
                   num_devices=NCORES)
    aps = {
        "xT_in": nc.dram_tensor("xT_in", [8, 128, NL], BF16,
                                kind="ExternalInput").ap(),
        "wqkvT_in": nc.dram_tensor("wqkvT_in", [8, 128, 3 * D], BF16,
                                   kind="ExternalInput").ap(),
        "wpT_in": nc.dram_tensor("wpT_in", [8, 128, D], BF16,
                                 kind="ExternalInput").ap(),
        "b_proj": nc.dram_tensor("b_proj", [D], F32, kind="ExternalInput").ap(),
        "out": nc.dram_tensor("out", [NL, D], BF16,
                              kind="ExternalOutput").ap(),
        "cc_kA": nc.dram_tensor("cc_kA", [512, NL], BF16).ap(),
        "cc_kB": nc.dram_tensor("cc_kB", [512, NL], BF16).ap(),
        "cc_vA": nc.dram_tensor("cc_vA", [512, D], BF16).ap(),
        "cc_vB": nc.dram_tensor("cc_vB", [512, D], BF16).ap(),
        "k_gA": nc.dram_tensor("k_gA", [2, 512, NL], BF16).ap(),
        "k_gB": nc.dram_tensor("k_gB", [2, 512, NL], BF16).ap(),
        "v_gA": nc.dram_tensor("v_gA", [2, 512, D], BF16).ap(),
        "v_gB": nc.dram_tensor("v_gB", [2, 512, D], BF16).ap(),
        "rc_d": nc.dram_tensor("rc_d", [8, 2, 2, 512], F32).ap(),
    }
    with tile.TileContext(nc) as tc:
        _emit(tc, aps)
    nc.compile()
    return nc


_NC = None


def _get_nc():
    global _NC
    if _NC is None:
        _NC = _build()
    return _NC


def run(x, w_qkv, w_proj, b_proj, **spmd_kwargs):
    nc = _get_nc()
    x = np.asarray(x, dtype=np.float32)
    w_qkv = np.asarray(w_qkv, dtype=np.float32)
    w_proj = np.asarray(w_proj, dtype=np.float32)
    b_proj = np.ascontiguousarray(np.asarray(b_proj, dtype=np.float32))
    wqkvT = np.ascontiguousarray(
        w_qkv.T.reshape(8, 128, 3 * D).astype(BF))
    wpT = np.ascontiguousarray(
        w_proj.T.reshape(8, 128, D).astype(BF))
    in_maps = []
    for c in range(NCORES):
        b, half = divmod(c, 2)
        xT = np.ascontiguousarray(
            x[b, half * NL:(half + 1) * NL, :].T.reshape(8, 128, NL).astype(BF))
        in_maps.append({
            "xT_in": xT,
            "wqkvT_in": wqkvT,
            "wpT_in": wpT,
            "b_proj": b_proj,
        })
    res = run_bass_kernel_spmd(nc, in_maps, list(range(NCORES)), **spmd_kwargs)
    y = np.empty((B, N, D), dtype=np.float32)
    for c in range(NCORES):
        b, half = divmod(c, 2)
        y[b, half * NL:(half + 1) * NL, :] = res.results[c]["out"].astype(
            np.float32)
    return y, res


def kernel(x, w_qkv, w_proj, b_proj):
    y, _ = run(x, w_qkv, w_proj, b_proj)
    return y
